# revision 8
# baseline (speedup 1.0000x reference)
"""DecompGridv3 embedding lookup on 8 Trainium2 NeuronCores.

Strategy (data-parallel over B=1M query points, 128K/core):
- Host prepares gather-friendly tables (one-time layout transform of weights):
    * grid_t:   (z,y,x,f) transposed 3D grid, rows of 32 f32 (128 B).
    * planes:   y-pair shingled: brick(y,x) = [P[y,x,:], P[y+1,x,:]] (64 f32).
      A query's 2x2 patch = bricks (y0,x0),(y0,x0+1) -> ONE contiguous 128-f32
      overlap-read per plane.
    * line:     pair-shingled [line_t[i], line_t[min(i+1,255)]] (64 f32).
- Device (per core): compute indices + interp weights on DVE, gather corners
  with gpsimd indirect DMA (grid: 4 x-pair reads/pt; planes: 1 brick-pair
  read/plane/pt) and a batched dma_gather for the line, then weighted-sum
  (tensor_tensor mult + segmented reduce) and the final 5-way product.
"""

import numpy as np

import concourse.bacc as bacc
import concourse.tile as tile
import concourse.mybir as mybir
from concourse.bass import AP, IndirectOffsetOnAxis
from concourse.bass_utils import run_bass_kernel_spmd

F32 = mybir.dt.float32
I32 = mybir.dt.int32
I16 = mybir.dt.int16
ALU = mybir.AluOpType

NF = 32          # features
D3 = 128         # 3D grid res
P2 = 384         # plane res
L1 = 256         # line length
B = 1 << 20      # total points
NCORES = 8
BCORE = B // NCORES          # 131072 points per core
JTOT = BCORE // 128          # 1024 free-dim point columns per core
CHUNK_J = 64                 # j-columns per chunk
NCHUNK = JTOT // CHUNK_J     # 16
TILE_S = 4                   # j-columns per compute tile
NTILE = CHUNK_J // TILE_S    # 8 tiles per chunk

# combined table layout (rows of 32 f32)
GRID_ROWS = D3 * D3 * D3                # 2097152 logical grid rows
GRID_TROWS = 2 * GRID_ROWS              # y-pair shingled
PLANE_BRICKS = P2 * P2                  # 147456 bricks of 2 rows
PLANE_ROWS = 2 * PLANE_BRICKS           # 294912
BASE_G = 0
BASE_P01 = GRID_TROWS
BASE_P02 = BASE_P01 + PLANE_ROWS
BASE_P12 = BASE_P02 + PLANE_ROWS
TAB_ROWS = BASE_P12 + PLANE_ROWS        # 5079040


def _ins0(ap: AP, pos: int, count: int) -> AP:
    """Insert a broadcast (step-0) dim at `pos` of ap's dim list."""
    dims = [list(d) for d in ap.ap]
    dims.insert(pos, [0, count])
    return AP(ap.tensor, ap.offset, dims)


def build_bass():
    nc = bacc.Bacc("TRN2", target_bir_lowering=False, debug=False,
                   num_devices=NCORES)
    xin = nc.dram_tensor("xin", [128, JTOT * 4], F32, kind="ExternalInput")
    tab = nc.dram_tensor("tab", [TAB_ROWS, NF], F32, kind="ExternalInput")
    ltab = nc.dram_tensor("ltab", [L1, 2 * NF], F32, kind="ExternalInput")
    xl16 = nc.dram_tensor("xl16", [16, JTOT * 8], F32, kind="ExternalInput")
    out = nc.dram_tensor("out", [128, JTOT * NF], F32, kind="ExternalOutput")

    J = CHUNK_J
    with tile.TileContext(nc) as tc:
        import contextlib
        with contextlib.ExitStack() as ctx:
            xp = ctx.enter_context(tc.tile_pool(name="xp", bufs=2))
            wp = ctx.enter_context(tc.tile_pool(name="wp", bufs=2))
            sp = ctx.enter_context(tc.tile_pool(name="sp", bufs=2))
            op = ctx.enter_context(tc.tile_pool(name="op", bufs=2))
            gp = ctx.enter_context(tc.tile_pool(name="gp", bufs=3))
            mp = ctx.enter_context(tc.tile_pool(name="mp", bufs=2))
            rp = ctx.enter_context(tc.tile_pool(name="rp", bufs=2))
            lp = ctx.enter_context(tc.tile_pool(name="lp", bufs=2))
            lip = ctx.enter_context(tc.tile_pool(name="lip", bufs=1))

            for c in range(NCHUNK):
                j0 = c * J
                # ---- load x chunk: [128, J, 4]
                xs = xp.tile([128, J, 4], F32, tag="xs")
                nc.sync.dma_start(
                    xs[:], xin.ap()[:, j0 * 4:(j0 + J) * 4]
                           .rearrange("p (j c) -> p j c", c=4))

                def coord(k):
                    return xs[:, :, k:k + 1].rearrange("p j o -> p (j o)")

                # ---- per-coord floors and fracs
                def floorfrac(fv, tg):
                    # fv: [128, J] f32 >= 0 -> (floor f32, frac f32)
                    ri = sp.tile([128, J], I32, tag="ffi", name="ri")
                    nc.vector.tensor_copy(ri[:], fv[:])          # round
                    rf = sp.tile([128, J], F32, tag="ffr", name="rf")
                    nc.vector.tensor_copy(rf[:], ri[:])
                    m = sp.tile([128, J], F32, tag="ffm", name="m")
                    nc.vector.tensor_tensor(out=m[:], in0=rf[:], in1=fv[:],
                                            op=ALU.is_gt)
                    fl = sp.tile([128, J], F32, tag=tg + "l", name="fl")
                    nc.vector.tensor_sub(fl[:], rf[:], m[:])
                    w = sp.tile([128, J], F32, tag=tg + "w", name="w")
                    nc.vector.tensor_sub(w[:], fv[:], fl[:])
                    return fl, w

                fl3, w3, fl2, w2 = [], [], [], []
                for k in range(3):
                    t = sp.tile([128, J], F32, tag="t")
                    nc.vector.tensor_scalar(out=t[:], in0=coord(k),
                                            scalar1=1.0, scalar2=0.5,
                                            op0=ALU.add, op1=ALU.mult)
                    f3 = sp.tile([128, J], F32, tag="f3")
                    nc.vector.tensor_scalar(out=f3[:], in0=t[:],
                                            scalar1=float(D3 - 1), scalar2=None,
                                            op0=ALU.mult)
                    f2 = sp.tile([128, J], F32, tag="f2")
                    nc.vector.tensor_scalar(out=f2[:], in0=t[:],
                                            scalar1=float(P2 - 1), scalar2=None,
                                            op0=ALU.mult)
                    a, b_ = floorfrac(f3, f"f3{k}")
                    fl3.append(a); w3.append(b_)
                    a, b_ = floorfrac(f2, f"f2{k}")
                    fl2.append(a); w2.append(b_)

                # line (for weights only; int16 idx handled separately)
                flv = sp.tile([128, J], F32, tag="flv")
                nc.vector.tensor_scalar(out=flv[:], in0=coord(3),
                                        scalar1=float(L1), scalar2=None,
                                        op0=ALU.mult)
                _, wl = floorfrac(flv, "fl3x")

                # ---- gather offsets (fp32 -> int32), in 32-f32-row units
                offg = op.tile([128, 2 * J], I32, tag="offg")
                b_ = sp.tile([128, J], F32, tag="gb")
                nc.vector.tensor_scalar(out=b_[:], in0=fl3[1],
                                        scalar1=float(D3), scalar2=None,
                                        op0=ALU.mult)
                a_ = sp.tile([128, J], F32, tag="ga")
                nc.vector.scalar_tensor_tensor(
                    out=a_[:], in0=fl3[2], scalar=float(D3 * D3), in1=b_[:],
                    op0=ALU.mult, op1=ALU.add)
                g00 = sp.tile([128, J], F32, tag="g00")
                nc.vector.tensor_add(g00[:], a_[:], fl3[0])
                for t_i in range(2):
                    gt = sp.tile([128, J], F32, tag="gt")
                    nc.vector.tensor_scalar(
                        out=gt[:], in0=g00[:], scalar1=2.0,
                        scalar2=float(t_i * 2 * D3 * D3),
                        op0=ALU.mult, op1=ALU.add)
                    nc.vector.tensor_copy(offg[:, t_i * J:(t_i + 1) * J], gt[:])

                offp = op.tile([128, 3 * J], I32, tag="offp")
                for p_i, (ky, kx, base) in enumerate(
                        ((1, 0, BASE_P01), (2, 0, BASE_P02), (2, 1, BASE_P12))):
                    r_ = sp.tile([128, J], F32, tag="pr")
                    nc.vector.scalar_tensor_tensor(
                        out=r_[:], in0=fl2[ky], scalar=float(P2), in1=fl2[kx],
                        op0=ALU.mult, op1=ALU.add)
                    r2 = sp.tile([128, J], F32, tag="pr2")
                    nc.vector.tensor_scalar(out=r2[:], in0=r_[:],
                                            scalar1=2.0, scalar2=float(base),
                                            op0=ALU.mult, op1=ALU.add)
                    nc.vector.tensor_copy(offp[:, p_i * J:(p_i + 1) * J], r2[:])

                # ---- weight pairs and corner coefficient products
                def mkpair(w, tag):
                    pr = wp.tile([128, J, 2], F32, tag=tag)
                    nc.vector.tensor_copy(pr[:, :, 1:2],
                                          w[:].to_broadcast([128, J, 1]))
                    nc.vector.tensor_scalar(
                        out=pr[:, :, 0:1],
                        in0=w[:].to_broadcast([128, J, 1]),
                        scalar1=1.0, scalar2=-1.0,
                        op0=ALU.subtract, op1=ALU.mult)
                    return pr

                wzp = mkpair(w3[2], "wzp")
                wyp = mkpair(w3[1], "wyp")
                wxp = mkpair(w3[0], "wxp")
                w2xp = mkpair(w2[0], "w2xp")
                w2yp = mkpair(w2[1], "w2yp")
                w2zp = mkpair(w2[2], "w2zp")
                wlp = mkpair(wl, "wlp")

                def outer2(pa, pb, tag):
                    # out[p, j, a, b] = pa[p,j,a] * pb[p,j,b]  -> [128, J, 4]
                    o = wp.tile([128, J, 2, 2], F32, tag=tag)
                    nc.vector.tensor_mul(
                        o[:], pa[:].to_broadcast([128, J, 2, 2]),
                        _ins0(pb[:], 2, 2))
                    return o

                wzx = outer2(wzp, wxp, "wzx")          # [p,J,(dz,dx)]
                # W8[p, J, (dz dx), dy] = wzx * wyp
                w8 = wp.tile([128, J, 8], F32, tag="w8")
                for dy in range(2):
                    w8ap = w8[:]
                    o_ap = AP(w8ap.tensor, w8ap.offset + dy,
                              [list(w8ap.ap[0]), [8, J], [2, 4]])
                    y_ap = wyp[:]
                    i_ap = AP(y_ap.tensor, y_ap.offset + dy,
                              [list(y_ap.ap[0]), [2, J], [0, 4]])
                    nc.vector.tensor_mul(o_ap,
                                         wzx[:].rearrange("p j a b -> p j (a b)"),
                                         i_ap)
                # plane coeffs: brick layout (dx, dy) -> W4[p,J,(dx,dy)]
                w401 = outer2(w2xp, w2yp, "w401")
                w402 = outer2(w2xp, w2zp, "w402")
                w412 = outer2(w2yp, w2zp, "w412")

                # ---- line: batched dma_gather for this chunk
                # int16 idx computed on partitions 0..15 from xl16 input
                # xl16 layout: xl16[q, c*8J + j*8 + k] = x3 of point
                # (p=16k+q, j0+j); value already scaled+floored? no: raw x3.
                li_f = lip.tile([16, J * 8], F32, tag="lif")
                nc.sync.dma_start(
                    li_f[:], xl16.ap()[:, c * J * 8:(c + 1) * J * 8])
                lfv = lip.tile([16, J * 8], F32, tag="lfv")
                nc.vector.tensor_scalar(out=lfv[:], in0=li_f[:],
                                        scalar1=float(L1), scalar2=None,
                                        op0=ALU.mult)
                lri = lip.tile([16, J * 8], I32, tag="lri")
                nc.vector.tensor_copy(lri[:], lfv[:])
                lrf = lip.tile([16, J * 8], F32, tag="lrf")
                nc.vector.tensor_copy(lrf[:], lri[:])
                lm = lip.tile([16, J * 8], F32, tag="lm")
                nc.vector.tensor_tensor(out=lm[:], in0=lrf[:], in1=lfv[:],
                                        op=ALU.is_gt)
                lfl = lip.tile([16, J * 8], F32, tag="lfl")
                nc.vector.tensor_sub(lfl[:], lrf[:], lm[:])
                lidx = lip.tile([128, J * 8], I16, tag="lidx")
                nc.vector.tensor_copy(lidx[0:16, :], lfl[:])
                # replicate to the other 7 16-partition groups
                for grp in range(1, 8):
                    nc.sync.dma_start(lidx[16 * grp:16 * (grp + 1), :],
                                      lidx[0:16, :])
                ld = lp.tile([128, J, 2 * NF], F32, tag="ld")
                nc.gpsimd.dma_gather(
                    out_ap=ld[:], in_ap=ltab.ap(), idxs_ap=lidx[:],
                    num_idxs=J * 128, num_idxs_reg=J * 128,
                    elem_size=2 * NF, single_packet=False)

                # ---- gathers + compute per tile
                for s in range(NTILE):
                    u0 = s * TILE_S
                    g3 = gp.tile([128, TILE_S, 2 * 4 * NF], F32, tag="g3")
                    gpl = [gp.tile([128, TILE_S, 4 * NF], F32, tag=f"gp{i}", name=f"gp{i}")
                           for i in range(3)]
                    for u in range(TILE_S):
                        j = u0 + u
                        for t_i in range(2):
                            nc.gpsimd.indirect_dma_start(
                                out=g3[:, u:u + 1, t_i * 128:(t_i + 1) * 128]
                                    .rearrange("p a b -> p (a b)"),
                                out_offset=None, in_=tab.ap(),
                                in_offset=IndirectOffsetOnAxis(
                                    ap=offg[:, t_i * J + j:t_i * J + j + 1],
                                    axis=0))
                        for p_i in range(3):
                            nc.gpsimd.indirect_dma_start(
                                out=gpl[p_i][:, u:u + 1, :]
                                    .rearrange("p a b -> p (a b)"),
                                out_offset=None, in_=tab.ap(),
                                in_offset=IndirectOffsetOnAxis(
                                    ap=offp[:, p_i * J + j:p_i * J + j + 1],
                                    axis=0))

                    # weighted sums
                    def wsum(gt, wt, ncorn, tag):
                        # gt view [p,S,(c f)]; wt [p, J, ncorn] slice [u0:u0+S]
                        m = mp.tile([128, TILE_S, NF, ncorn], F32, tag=tag)
                        nc.vector.tensor_mul(
                            m[:].rearrange("p u f c -> p u c f"),
                            gt.rearrange("p u (c f) -> p u c f", c=ncorn, f=NF),
                            wt[:, u0:u0 + TILE_S, :]
                                .to_broadcast([128, TILE_S, ncorn, NF]))
                        r = rp.tile([128, TILE_S, NF], F32, tag=tag + "r")
                        nc.vector.tensor_reduce(out=r[:], in_=m[:],
                                                axis=mybir.AxisListType.X,
                                                op=ALU.add)
                        return r

                    f3r = wsum(g3[:], w8, 8, "m8")
                    p01r = wsum(gpl[0][:], w401[:].rearrange(
                        "p j a b -> p j (a b)"), 4, "m01")
                    p02r = wsum(gpl[1][:], w402[:].rearrange(
                        "p j a b -> p j (a b)"), 4, "m02")
                    p12r = wsum(gpl[2][:], w412[:].rearrange(
                        "p j a b -> p j (a b)"), 4, "m12")
                    flr = wsum(ld[:, u0:u0 + TILE_S, :], wlp[:].rearrange(
                        "p j a -> p j a"), 2, "ml")

                    o1 = rp.tile([128, TILE_S, NF], F32, tag="o1")
                    nc.vector.tensor_mul(o1[:], f3r[:], p01r[:])
                    o2 = rp.tile([128, TILE_S, NF], F32, tag="o2")
                    nc.vector.tensor_mul(o2[:], p02r[:], p12r[:])
                    o3 = rp.tile([128, TILE_S, NF], F32, tag="o3")
                    nc.vector.tensor_mul(o3[:], o1[:], o2[:])
                    ot = rp.tile([128, TILE_S, NF], F32, tag="ot")
                    nc.vector.tensor_mul(ot[:], o3[:], flr[:])
                    nc.sync.dma_start(
                        out.ap()[:, (j0 + u0) * NF:(j0 + u0 + TILE_S) * NF],
                        ot[:].rearrange("p u f -> p (u f)"))

    nc.compile()
    return nc


def _prep_tables(grid3d, plane01, plane02, plane12, line0):
    gt = np.ascontiguousarray(
        grid3d.transpose(1, 2, 3, 0)).reshape(GRID_ROWS, NF)
    gy = np.empty((GRID_ROWS, 2, NF), np.float32)
    gy[:, 0] = gt
    gy[:-D3, 1] = gt[D3:]
    gy[-D3:, 1] = gt[-D3:]
    grid_t = gy.reshape(GRID_TROWS, NF)
    del gy

    def shingle_plane(p):
        pt = np.ascontiguousarray(p.transpose(1, 2, 0))      # (y, x, f)
        ps = np.empty((P2, P2, 2, NF), np.float32)
        ps[:, :, 0, :] = pt
        ps[:-1, :, 1, :] = pt[1:]
        ps[-1, :, 1, :] = pt[-1]
        return ps.reshape(PLANE_ROWS, NF)

    tab = np.concatenate([grid_t, shingle_plane(plane01),
                          shingle_plane(plane02), shingle_plane(plane12)],
                         axis=0)
    lt = np.ascontiguousarray(line0.T)                        # (256, 32)
    ls = np.empty((L1, 2 * NF), np.float32)
    ls[:, :NF] = lt
    ls[:-1, NF:] = lt[1:]
    ls[-1, NF:] = lt[-1]
    return tab, ls


_NC_CACHE = {}


def kernel(x, grid3d, plane01, plane02, plane12, line0):
    x = np.asarray(x, np.float32)
    tab, ls = _prep_tables(np.asarray(grid3d, np.float32),
                           np.asarray(plane01, np.float32),
                           np.asarray(plane02, np.float32),
                           np.asarray(plane12, np.float32),
                           np.asarray(line0, np.float32))
    if "nc" not in _NC_CACHE:
        _NC_CACHE["nc"] = build_bass()
    nc = _NC_CACHE["nc"]

    in_maps = []
    for cix in range(NCORES):
        xc = x[cix * BCORE:(cix + 1) * BCORE]          # (131072, 4)
        xin = np.ascontiguousarray(xc.reshape(128, JTOT * 4))
        # xl16[q, c*8J + j*8 + k] = x3 of point (p=16k+q, jglob=c*J+j)
        x3 = xc[:, 3].reshape(128, JTOT)               # [p, jglob]
        x3g = x3.reshape(8, 16, NCHUNK, CHUNK_J)       # [k, q, c, j]
        xl16 = np.ascontiguousarray(
            x3g.transpose(1, 2, 3, 0).reshape(16, JTOT * 8))
        in_maps.append({"xin": xin, "tab": tab, "ltab": ls, "xl16": xl16})

    res = run_bass_kernel_spmd(nc, in_maps, core_ids=list(range(NCORES)))
    outs = [r["out"].reshape(BCORE, NF) for r in res.results]
    return np.concatenate(outs, axis=0)


# revision 11
# speedup vs baseline: 1.8004x; 1.8004x over previous
"""DecompGridv3 embedding lookup on 8 Trainium2 NeuronCores.

Strategy (data-parallel over B=1M query points, 128K/core):
- Host prepares gather-friendly tables (one-time layout transform of weights):
    * grid_t:   (z,y,x,f) transposed 3D grid, rows of 32 f32 (128 B).
    * planes:   y-pair shingled: brick(y,x) = [P[y,x,:], P[y+1,x,:]] (64 f32).
      A query's 2x2 patch = bricks (y0,x0),(y0,x0+1) -> ONE contiguous 128-f32
      overlap-read per plane.
    * line:     pair-shingled [line_t[i], line_t[min(i+1,255)]] (64 f32).
- Device (per core): compute indices + interp weights on DVE, gather corners
  with gpsimd indirect DMA (grid: 4 x-pair reads/pt; planes: 1 brick-pair
  read/plane/pt) and a batched dma_gather for the line, then weighted-sum
  (tensor_tensor mult + segmented reduce) and the final 5-way product.
"""

import numpy as np

import concourse.bacc as bacc
import concourse.tile as tile
import concourse.mybir as mybir
from concourse.bass import AP, IndirectOffsetOnAxis
from concourse.bass_utils import run_bass_kernel_spmd

F32 = mybir.dt.float32
I32 = mybir.dt.int32
I16 = mybir.dt.int16
ALU = mybir.AluOpType

NF = 32          # features
D3 = 128         # 3D grid res
P2 = 384         # plane res
L1 = 256         # line length
B = 1 << 20      # total points
NCORES = 8
BCORE = B // NCORES          # 131072 points per core
JTOT = BCORE // 128          # 1024 free-dim point columns per core
CHUNK_J = 64                 # j-columns per chunk
NCHUNK = JTOT // CHUNK_J     # 16
TILE_S = 4                   # j-columns per compute tile
REPEAT = 1                   # in-kernel repeat (timing only)
NTILE = CHUNK_J // TILE_S    # 8 tiles per chunk

# combined table layout (rows of 32 f32)
GRID_ROWS = D3 * D3 * D3                # 2097152 logical grid rows
GRID_TROWS = 2 * GRID_ROWS              # y-pair shingled
PLANE_BRICKS = P2 * P2                  # 147456 bricks of 2 rows
PLANE_ROWS = 2 * PLANE_BRICKS           # 294912
BASE_G = 0
BASE_P01 = GRID_TROWS
BASE_P02 = BASE_P01 + PLANE_ROWS
BASE_P12 = BASE_P02 + PLANE_ROWS
TAB_ROWS = BASE_P12 + PLANE_ROWS        # 5079040


def _ins0(ap: AP, pos: int, count: int) -> AP:
    """Insert a broadcast (step-0) dim at `pos` of ap's dim list."""
    dims = [list(d) for d in ap.ap]
    dims.insert(pos, [0, count])
    return AP(ap.tensor, ap.offset, dims)


def build_bass():
    nc = bacc.Bacc("TRN2", target_bir_lowering=False, debug=False,
                   num_devices=NCORES)
    xin = nc.dram_tensor("xin", [128, JTOT * 4], F32, kind="ExternalInput")
    tab = nc.dram_tensor("tab", [TAB_ROWS, NF], F32, kind="ExternalInput")
    ltab = nc.dram_tensor("ltab", [L1, 2 * NF], F32, kind="ExternalInput")
    xl16 = nc.dram_tensor("xl16", [16, JTOT * 8], F32, kind="ExternalInput")
    out = nc.dram_tensor("out", [128, JTOT * NF], F32, kind="ExternalOutput")

    J = CHUNK_J
    with tile.TileContext(nc) as tc:
        import contextlib
        with contextlib.ExitStack() as ctx:
            xp = ctx.enter_context(tc.tile_pool(name="xp", bufs=2))
            wp = ctx.enter_context(tc.tile_pool(name="wp", bufs=2))
            sp = ctx.enter_context(tc.tile_pool(name="sp", bufs=2))
            op = ctx.enter_context(tc.tile_pool(name="op", bufs=3))
            gp = ctx.enter_context(tc.tile_pool(name="gp", bufs=6))
            mp = ctx.enter_context(tc.tile_pool(name="mp", bufs=2))
            rp = ctx.enter_context(tc.tile_pool(name="rp", bufs=2))
            lp = ctx.enter_context(tc.tile_pool(name="lp", bufs=3))
            lip = ctx.enter_context(tc.tile_pool(name="lip", bufs=2))

            rep_ctx = (tc.For_i(0, REPEAT, 1) if REPEAT > 1
                       else contextlib.nullcontext())
            with rep_ctx:
              for c in range(NCHUNK):
                j0 = c * J
                # ---- load x chunk: [128, J, 4]
                xs = xp.tile([128, J, 4], F32, tag="xs")
                nc.sync.dma_start(
                    xs[:], xin.ap()[:, j0 * 4:(j0 + J) * 4]
                           .rearrange("p (j c) -> p j c", c=4))

                def coord(k):
                    return xs[:, :, k:k + 1].rearrange("p j o -> p (j o)")

                # ---- per-coord floors and fracs
                def floorfrac(fv, tg):
                    # fv: [128, J] f32 >= 0 -> (floor f32, frac f32)
                    ri = sp.tile([128, J], I32, tag="ffi", name="ri")
                    nc.vector.tensor_copy(ri[:], fv[:])          # round
                    rf = sp.tile([128, J], F32, tag="ffr", name="rf")
                    nc.vector.tensor_copy(rf[:], ri[:])
                    m = sp.tile([128, J], F32, tag="ffm", name="m")
                    nc.vector.tensor_tensor(out=m[:], in0=rf[:], in1=fv[:],
                                            op=ALU.is_gt)
                    fl = sp.tile([128, J], F32, tag=tg + "l", name="fl")
                    nc.vector.tensor_sub(fl[:], rf[:], m[:])
                    w = sp.tile([128, J], F32, tag=tg + "w", name="w")
                    nc.vector.tensor_sub(w[:], fv[:], fl[:])
                    return fl, w

                fl3, w3, fl2, w2 = [], [], [], []
                for k in range(3):
                    t = sp.tile([128, J], F32, tag="t")
                    nc.vector.tensor_scalar(out=t[:], in0=coord(k),
                                            scalar1=1.0, scalar2=0.5,
                                            op0=ALU.add, op1=ALU.mult)
                    f3 = sp.tile([128, J], F32, tag="f3")
                    nc.vector.tensor_scalar(out=f3[:], in0=t[:],
                                            scalar1=float(D3 - 1), scalar2=None,
                                            op0=ALU.mult)
                    f2 = sp.tile([128, J], F32, tag="f2")
                    nc.vector.tensor_scalar(out=f2[:], in0=t[:],
                                            scalar1=float(P2 - 1), scalar2=None,
                                            op0=ALU.mult)
                    a, b_ = floorfrac(f3, f"f3{k}")
                    fl3.append(a); w3.append(b_)
                    a, b_ = floorfrac(f2, f"f2{k}")
                    fl2.append(a); w2.append(b_)

                # line (for weights only; int16 idx handled separately)
                flv = sp.tile([128, J], F32, tag="flv")
                nc.vector.tensor_scalar(out=flv[:], in0=coord(3),
                                        scalar1=float(L1), scalar2=None,
                                        op0=ALU.mult)
                _, wl = floorfrac(flv, "fl3x")

                # ---- gather offsets (fp32 -> int32), in 32-f32-row units
                offg = op.tile([128, 2 * J], I32, tag="offg")
                b_ = sp.tile([128, J], F32, tag="gb")
                nc.vector.tensor_scalar(out=b_[:], in0=fl3[1],
                                        scalar1=float(D3), scalar2=None,
                                        op0=ALU.mult)
                a_ = sp.tile([128, J], F32, tag="ga")
                nc.vector.scalar_tensor_tensor(
                    out=a_[:], in0=fl3[2], scalar=float(D3 * D3), in1=b_[:],
                    op0=ALU.mult, op1=ALU.add)
                g00 = sp.tile([128, J], F32, tag="g00")
                nc.vector.tensor_add(g00[:], a_[:], fl3[0])
                for t_i in range(2):
                    gt = sp.tile([128, J], F32, tag="gt")
                    nc.vector.tensor_scalar(
                        out=gt[:], in0=g00[:], scalar1=2.0,
                        scalar2=float(t_i * 2 * D3 * D3),
                        op0=ALU.mult, op1=ALU.add)
                    nc.vector.tensor_copy(offg[:, t_i * J:(t_i + 1) * J], gt[:])

                offp = op.tile([128, 3 * J], I32, tag="offp")
                for p_i, (ky, kx, base) in enumerate(
                        ((1, 0, BASE_P01), (2, 0, BASE_P02), (2, 1, BASE_P12))):
                    r_ = sp.tile([128, J], F32, tag="pr")
                    nc.vector.scalar_tensor_tensor(
                        out=r_[:], in0=fl2[ky], scalar=float(P2), in1=fl2[kx],
                        op0=ALU.mult, op1=ALU.add)
                    r2 = sp.tile([128, J], F32, tag="pr2")
                    nc.vector.tensor_scalar(out=r2[:], in0=r_[:],
                                            scalar1=2.0, scalar2=float(base),
                                            op0=ALU.mult, op1=ALU.add)
                    nc.vector.tensor_copy(offp[:, p_i * J:(p_i + 1) * J], r2[:])

                # ---- weight pairs and corner coefficient products
                def mkpair(w, tag):
                    pr = wp.tile([128, J, 2], F32, tag=tag)
                    nc.vector.tensor_copy(pr[:, :, 1:2],
                                          w[:].to_broadcast([128, J, 1]))
                    nc.vector.tensor_scalar(
                        out=pr[:, :, 0:1],
                        in0=w[:].to_broadcast([128, J, 1]),
                        scalar1=1.0, scalar2=-1.0,
                        op0=ALU.subtract, op1=ALU.mult)
                    return pr

                wzp = mkpair(w3[2], "wzp")
                wyp = mkpair(w3[1], "wyp")
                wxp = mkpair(w3[0], "wxp")
                w2xp = mkpair(w2[0], "w2xp")
                w2yp = mkpair(w2[1], "w2yp")
                w2zp = mkpair(w2[2], "w2zp")
                wlp = mkpair(wl, "wlp")

                def outer2(pa, pb, tag):
                    # out[p, j, a, b] = pa[p,j,a] * pb[p,j,b]  -> [128, J, 4]
                    o = wp.tile([128, J, 2, 2], F32, tag=tag)
                    nc.vector.tensor_mul(
                        o[:], pa[:].to_broadcast([128, J, 2, 2]),
                        _ins0(pb[:], 2, 2))
                    return o

                wzx = outer2(wzp, wxp, "wzx")          # [p,J,(dz,dx)]
                # W8[p, J, (dz dx), dy] = wzx * wyp
                w8 = wp.tile([128, J, 8], F32, tag="w8")
                for dy in range(2):
                    w8ap = w8[:]
                    o_ap = AP(w8ap.tensor, w8ap.offset + dy,
                              [list(w8ap.ap[0]), [8, J], [2, 4]])
                    y_ap = wyp[:]
                    i_ap = AP(y_ap.tensor, y_ap.offset + dy,
                              [list(y_ap.ap[0]), [2, J], [0, 4]])
                    nc.vector.tensor_mul(o_ap,
                                         wzx[:].rearrange("p j a b -> p j (a b)"),
                                         i_ap)
                # plane coeffs: brick layout (dx, dy) -> W4[p,J,(dx,dy)]
                w401 = outer2(w2xp, w2yp, "w401")
                w402 = outer2(w2xp, w2zp, "w402")
                w412 = outer2(w2yp, w2zp, "w412")

                # ---- line: batched dma_gather for this chunk
                # int16 idx computed on partitions 0..15 from xl16 input
                # xl16 layout: xl16[q, c*8J + j*8 + k] = x3 of point
                # (p=16k+q, j0+j); value already scaled+floored? no: raw x3.
                li_f = lip.tile([16, J * 8], F32, tag="lif")
                nc.sync.dma_start(
                    li_f[:], xl16.ap()[:, c * J * 8:(c + 1) * J * 8])
                lfv = lip.tile([16, J * 8], F32, tag="lfv")
                nc.vector.tensor_scalar(out=lfv[:], in0=li_f[:],
                                        scalar1=float(L1), scalar2=None,
                                        op0=ALU.mult)
                lri = lip.tile([16, J * 8], I32, tag="lri")
                nc.vector.tensor_copy(lri[:], lfv[:])
                lrf = lip.tile([16, J * 8], F32, tag="lrf")
                nc.vector.tensor_copy(lrf[:], lri[:])
                lm = lip.tile([16, J * 8], F32, tag="lm")
                nc.vector.tensor_tensor(out=lm[:], in0=lrf[:], in1=lfv[:],
                                        op=ALU.is_gt)
                lfl = lip.tile([16, J * 8], F32, tag="lfl")
                nc.vector.tensor_sub(lfl[:], lrf[:], lm[:])
                lidx = lip.tile([128, J * 8], I16, tag="lidx")
                nc.vector.tensor_copy(lidx[0:16, :], lfl[:])
                # replicate to the other 7 16-partition groups
                for grp in range(1, 8):
                    nc.sync.dma_start(lidx[16 * grp:16 * (grp + 1), :],
                                      lidx[0:16, :])
                ld = lp.tile([128, J, 2 * NF], F32, tag="ld")
                nc.gpsimd.dma_gather(
                    out_ap=ld[:], in_ap=ltab.ap(), idxs_ap=lidx[:],
                    num_idxs=J * 128, num_idxs_reg=J * 128,
                    elem_size=2 * NF, single_packet=False)

                # ---- gathers + compute per tile
                for s in range(NTILE):
                    u0 = s * TILE_S
                    g3 = gp.tile([128, TILE_S, 2 * 4 * NF], F32, tag="g3")
                    gpl = [gp.tile([128, TILE_S, 4 * NF], F32, tag=f"gp{i}", name=f"gp{i}")
                           for i in range(3)]
                    for u in range(TILE_S):
                        j = u0 + u
                        for t_i in range(2):
                            nc.gpsimd.indirect_dma_start(
                                out=g3[:, u:u + 1, t_i * 128:(t_i + 1) * 128]
                                    .rearrange("p a b -> p (a b)"),
                                out_offset=None, in_=tab.ap(),
                                in_offset=IndirectOffsetOnAxis(
                                    ap=offg[:, t_i * J + j:t_i * J + j + 1],
                                    axis=0))
                        for p_i in range(3):
                            nc.gpsimd.indirect_dma_start(
                                out=gpl[p_i][:, u:u + 1, :]
                                    .rearrange("p a b -> p (a b)"),
                                out_offset=None, in_=tab.ap(),
                                in_offset=IndirectOffsetOnAxis(
                                    ap=offp[:, p_i * J + j:p_i * J + j + 1],
                                    axis=0))

                    # weighted sums
                    def wsum(gt, wt, ncorn, tag):
                        # gt view [p,S,(c f)]; wt [p, J, ncorn] slice [u0:u0+S]
                        m = mp.tile([128, TILE_S, NF, ncorn], F32, tag=tag)
                        nc.vector.tensor_mul(
                            m[:].rearrange("p u f c -> p u c f"),
                            gt.rearrange("p u (c f) -> p u c f", c=ncorn, f=NF),
                            wt[:, u0:u0 + TILE_S, :]
                                .to_broadcast([128, TILE_S, ncorn, NF]))
                        r = rp.tile([128, TILE_S, NF], F32, tag=tag + "r")
                        nc.vector.tensor_reduce(out=r[:], in_=m[:],
                                                axis=mybir.AxisListType.X,
                                                op=ALU.add)
                        return r

                    f3r = wsum(g3[:], w8, 8, "m8")
                    p01r = wsum(gpl[0][:], w401[:].rearrange(
                        "p j a b -> p j (a b)"), 4, "m01")
                    p02r = wsum(gpl[1][:], w402[:].rearrange(
                        "p j a b -> p j (a b)"), 4, "m02")
                    p12r = wsum(gpl[2][:], w412[:].rearrange(
                        "p j a b -> p j (a b)"), 4, "m12")
                    flr = wsum(ld[:, u0:u0 + TILE_S, :], wlp[:].rearrange(
                        "p j a -> p j a"), 2, "ml")

                    o1 = rp.tile([128, TILE_S, NF], F32, tag="o1")
                    nc.vector.tensor_mul(o1[:], f3r[:], p01r[:])
                    o2 = rp.tile([128, TILE_S, NF], F32, tag="o2")
                    nc.vector.tensor_mul(o2[:], p02r[:], p12r[:])
                    o3 = rp.tile([128, TILE_S, NF], F32, tag="o3")
                    nc.vector.tensor_mul(o3[:], o1[:], o2[:])
                    ot = rp.tile([128, TILE_S, NF], F32, tag="ot")
                    nc.vector.tensor_mul(ot[:], o3[:], flr[:])
                    nc.sync.dma_start(
                        out.ap()[:, (j0 + u0) * NF:(j0 + u0 + TILE_S) * NF],
                        ot[:].rearrange("p u f -> p (u f)"))

    nc.compile()
    return nc


def _prep_tables(grid3d, plane01, plane02, plane12, line0):
    gt = np.ascontiguousarray(
        grid3d.transpose(1, 2, 3, 0)).reshape(GRID_ROWS, NF)
    gy = np.empty((GRID_ROWS, 2, NF), np.float32)
    gy[:, 0] = gt
    gy[:-D3, 1] = gt[D3:]
    gy[-D3:, 1] = gt[-D3:]
    grid_t = gy.reshape(GRID_TROWS, NF)
    del gy

    def shingle_plane(p):
        pt = np.ascontiguousarray(p.transpose(1, 2, 0))      # (y, x, f)
        ps = np.empty((P2, P2, 2, NF), np.float32)
        ps[:, :, 0, :] = pt
        ps[:-1, :, 1, :] = pt[1:]
        ps[-1, :, 1, :] = pt[-1]
        return ps.reshape(PLANE_ROWS, NF)

    tab = np.concatenate([grid_t, shingle_plane(plane01),
                          shingle_plane(plane02), shingle_plane(plane12)],
                         axis=0)
    lt = np.ascontiguousarray(line0.T)                        # (256, 32)
    ls = np.empty((L1, 2 * NF), np.float32)
    ls[:, :NF] = lt
    ls[:-1, NF:] = lt[1:]
    ls[-1, NF:] = lt[-1]
    return tab, ls


_NC_CACHE = {}


def kernel(x, grid3d, plane01, plane02, plane12, line0):
    x = np.asarray(x, np.float32)
    tab, ls = _prep_tables(np.asarray(grid3d, np.float32),
                           np.asarray(plane01, np.float32),
                           np.asarray(plane02, np.float32),
                           np.asarray(plane12, np.float32),
                           np.asarray(line0, np.float32))
    if "nc" not in _NC_CACHE:
        _NC_CACHE["nc"] = build_bass()
    nc = _NC_CACHE["nc"]

    in_maps = []
    for cix in range(NCORES):
        xc = x[cix * BCORE:(cix + 1) * BCORE]          # (131072, 4)
        xin = np.ascontiguousarray(xc.reshape(128, JTOT * 4))
        # xl16[q, c*8J + j*8 + k] = x3 of point (p=16k+q, jglob=c*J+j)
        x3 = xc[:, 3].reshape(128, JTOT)               # [p, jglob]
        x3g = x3.reshape(8, 16, NCHUNK, CHUNK_J)       # [k, q, c, j]
        xl16 = np.ascontiguousarray(
            x3g.transpose(1, 2, 3, 0).reshape(16, JTOT * 8))
        in_maps.append({"xin": xin, "tab": tab, "ltab": ls, "xl16": xl16})

    res = run_bass_kernel_spmd(nc, in_maps, core_ids=list(range(NCORES)))
    outs = [r["out"].reshape(BCORE, NF) for r in res.results]
    return np.concatenate(outs, axis=0)
